# revision 3
# baseline (speedup 1.0000x reference)
"""Trainium2 Bass kernel for nn_APPM_24111946399794 (nms_detection).

Per batch element (B=65536): 741 multi-scale VALID avgpool window scores from a
1x14x14 map, greedy NMS per 3 ratio groups (3+2+1 picks, IoU<=0.25), returns
(proposalN_indices [B,6] i32, proposalN_windows_scores [B,6] f32,
 window_scores [B,741] f32).

Device (8 NeuronCores, batch-sharded): fp32 matmul against a 0/1 pooling
matrix computes exact window SUMS (PE), plus per-ratio-block top-8
(value+index) extraction for NMS groups 0/1 (DVE max8/max_index).
Host: exact fp32 division by window area, candidate walk with precomputed
IoU-suppression tables, vectorized full-NMS fallback for batches whose picks
are not contained in the shipped candidates, and an exact sequential-order
recompute for near-tie batches.
"""

from contextlib import ExitStack

import numpy as np

import concourse.bacc as bacc
import concourse.mybir as mybir
import concourse.tile as tile
from concourse.bass_utils import run_bass_kernel_spmd
from concourse.masks import make_identity

# ---------------------------------------------------------------- problem spec
SIZE = 14
RATIOS = [(4, 4), (3, 5), (5, 3), (6, 6), (5, 7), (7, 5), (8, 8), (6, 10), (10, 6)]
GROUPS = [(0, 3), (3, 6), (6, 9)]
N_LIST = [3, 2, 1]
IOU_THRESHS = [0.25, 0.25, 0.25]
WIN_NUMS = [(SIZE - h + 1) * (SIZE - w + 1) for h, w in RATIOS]
OFFS = np.cumsum([0] + WIN_NUMS)          # [0,121,241,361,442,522,602,651,696,741]
NWIN = int(OFFS[-1])                      # 741
NCORES = 8
P = 128
NFEAT = SIZE * SIZE                       # 196
K2 = NFEAT - P                            # 68
# ratio blocks used for device-side top-8 extraction (groups 0 and 1)
EX_BLOCKS = [(int(OFFS[i]), int(OFFS[i + 1])) for i in range(6)]
NBLK = len(EX_BLOCKS)
NCAND = 8 * NBLK                          # 48 per batch element
EPS_TIE = 3e-5                            # near-tie flag threshold on sums

_KERNEL_CACHE: dict = {}
PROFILE = False
_LAST_RESULTS = None


def _build_M() -> np.ndarray:
    """[196, 741] 0/1 fp32 pooling-sum matrix; column order matches reference."""
    M = np.zeros((NFEAT, NWIN), np.float32)
    col = 0
    for (h, w) in RATIOS:
        for io in range(SIZE - h + 1):
            for jo in range(SIZE - w + 1):
                blk = np.zeros((SIZE, SIZE), np.float32)
                blk[io:io + h, jo:jo + w] = 1.0
                M[:, col] = blk.reshape(NFEAT)
                col += 1
    return M


def _hw_row() -> np.ndarray:
    """[741] fp32 window areas (h*w) per window for the final division."""
    return np.concatenate([
        np.full((WIN_NUMS[i],), np.float32(h * w), np.float32)
        for i, (h, w) in enumerate(RATIOS)
    ])


def _build_device_kernel(Bc: int):
    """Bass kernel for one core processing Bc batch rows."""
    assert Bc % P == 0
    ntiles = Bc // P
    nc = bacc.Bacc("TRN2", target_bir_lowering=False, debug=False)
    f32 = mybir.dt.float32
    u32 = mybir.dt.uint32

    x_d = nc.dram_tensor("x", [Bc, NFEAT], f32, kind="ExternalInput").ap()
    m0_d = nc.dram_tensor("m0", [P, NWIN], f32, kind="ExternalInput").ap()
    m1_d = nc.dram_tensor("m1", [K2, NWIN], f32, kind="ExternalInput").ap()
    sums_d = nc.dram_tensor("sums", [Bc, NWIN], f32, kind="ExternalOutput").ap()
    cv_d = nc.dram_tensor("cand_vals", [Bc, NCAND], f32, kind="ExternalOutput").ap()
    ci_d = nc.dram_tensor("cand_idx", [Bc, NCAND], u32, kind="ExternalOutput").ap()

    with tile.TileContext(nc) as tc, ExitStack() as ctx:
        const = ctx.enter_context(tc.tile_pool(name="const", bufs=1))
        xin = ctx.enter_context(tc.tile_pool(name="xin", bufs=4))
        xtp = ctx.enter_context(tc.tile_pool(name="xtp", bufs=2, space="PSUM"))
        xts = ctx.enter_context(tc.tile_pool(name="xts", bufs=2))
        sps = ctx.enter_context(tc.tile_pool(name="sps", bufs=2, space="PSUM"))
        ssb = ctx.enter_context(tc.tile_pool(name="ssb", bufs=3))
        cnd = ctx.enter_context(tc.tile_pool(name="cnd", bufs=3))

        m0_sb = const.tile([P, NWIN], f32)
        nc.sync.dma_start(m0_sb[:], m0_d)
        m1_sb = const.tile([K2, NWIN], f32)
        nc.sync.dma_start(m1_sb[:], m1_d)
        ident = const.tile([P, P], f32)
        make_identity(nc, ident[:])

        for t in range(ntiles):
            rows = slice(t * P, (t + 1) * P)

            x_sb = xin.tile([P, NFEAT], f32)
            nc.sync.dma_start(x_sb[:], x_d[rows, :])

            # transpose x -> xT (features on partitions), via PE + ACT copy
            xt_ps = xtp.tile([P, 2 * P], f32)
            nc.tensor.transpose(xt_ps[:, 0:P], x_sb[:, 0:P], ident[:])
            nc.tensor.transpose(xt_ps[0:K2, P:2 * P], x_sb[:, P:NFEAT], ident[:])
            xt0_sb = xts.tile([P, P], f32)
            nc.scalar.copy(xt0_sb[:], xt_ps[:, 0:P])
            xt1_sb = xts.tile([K2, P], f32)
            nc.scalar.copy(xt1_sb[:], xt_ps[0:K2, P:2 * P])

            # window sums: [128b, 741] = xT.T @ M, fp32, K split 128+68
            s0_ps = sps.tile([P, 512], f32)
            s1_ps = sps.tile([P, NWIN - 512], f32)
            nc.tensor.matmul(s0_ps[:], xt0_sb[:], m0_sb[:, 0:512], start=True, stop=False)
            nc.tensor.matmul(s0_ps[:], xt1_sb[:], m1_sb[:, 0:512], start=False, stop=True)
            nc.tensor.matmul(s1_ps[:], xt0_sb[:], m0_sb[:, 512:NWIN], start=True, stop=False)
            nc.tensor.matmul(s1_ps[:], xt1_sb[:], m1_sb[:, 512:NWIN], start=False, stop=True)

            sums_sb = ssb.tile([P, NWIN], f32)
            nc.scalar.copy(sums_sb[:, 0:512], s0_ps[:])
            nc.scalar.copy(sums_sb[:, 512:NWIN], s1_ps[:])

            # per-ratio-block top-8 values + indices (groups 0 and 1)
            cv_sb = cnd.tile([P, NCAND], f32)
            ci_sb = cnd.tile([P, NCAND], u32)
            for j, (s, e) in enumerate(EX_BLOCKS):
                nc.vector.max(out=cv_sb[:, 8 * j:8 * j + 8], in_=sums_sb[:, s:e])
                nc.vector.max_index(
                    out=ci_sb[:, 8 * j:8 * j + 8],
                    in_max=cv_sb[:, 8 * j:8 * j + 8],
                    in_values=sums_sb[:, s:e],
                )

            nc.sync.dma_start(sums_d[rows, :], sums_sb[:])
            nc.sync.dma_start(cv_d[rows, :], cv_sb[:])
            nc.sync.dma_start(ci_d[rows, :], ci_sb[:])

    nc.compile()
    return nc


def _get_kernel(Bc: int):
    if Bc not in _KERNEL_CACHE:
        _KERNEL_CACHE[Bc] = _build_device_kernel(Bc)
    return _KERNEL_CACHE[Bc]


# ---------------------------------------------------------------- host helpers

def _sup_tables(coords: np.ndarray):
    """Per-group boolean suppression tables S[i, j] = IoU(i, j) > thresh."""
    cf = coords.astype(np.float32)
    areas = (cf[:, 2] - cf[:, 0] + 1.0) * (cf[:, 3] - cf[:, 1] + 1.0)
    tabs = []
    for g, (a, b) in enumerate(GROUPS):
        s, e = int(OFFS[a]), int(OFFS[b])
        c = cf[s:e]
        ar = areas[s:e]
        lx = np.minimum(c[None, :, 2], c[:, None, 2]) - np.maximum(c[None, :, 0], c[:, None, 0]) + 1.0
        ly = np.minimum(c[None, :, 3], c[:, None, 3]) - np.maximum(c[None, :, 1], c[:, None, 1]) + 1.0
        inter = np.where((lx < 0) | (ly < 0), 0.0, lx * ly)
        iou = inter / (ar[None, :] + ar[:, None] - inter)
        tabs.append(iou > IOU_THRESHS[g])
    return tabs


def _full_nms_group(scores_g: np.ndarray, S: np.ndarray, N: int):
    """Vectorized greedy NMS over a [n, W] slice. Returns picks [n, N] and the
    min argmax gap (top1 - top2 of the masked array at each step) [n]."""
    n = scores_g.shape[0]
    cur = scores_g.astype(np.float32).copy()
    picks = np.empty((n, N), np.int64)
    mingap = np.full((n,), np.inf, np.float32)
    rng = np.arange(n)
    for k in range(N):
        p = np.argmax(cur, axis=1)
        picks[:, k] = p
        pv = cur[rng, p]
        cur[rng, p] = -np.inf
        runner = np.max(cur, axis=1)
        gap = pv - runner
        np.minimum(mingap, gap, out=mingap)
        if k + 1 < N:
            cur[S[p]] = -np.inf
        else:
            cur[rng, p] = pv  # restore, not needed further but keep tidy
    return picks, mingap


def _exact_scores_from_x(xf: np.ndarray) -> np.ndarray:
    """Reference-bitexact scores for a (small) batch subset: sequential fp32
    accumulation over window elements in row-major order, then fp32 divide."""
    M = _build_M()
    n = xf.shape[0]
    acc = np.zeros((n, NWIN), np.float32)
    for k in range(NFEAT):
        acc += xf[:, k:k + 1] * M[k][None, :]
    return acc / _hw_row()[None, :]


def _host_nms(all_scores, sums, cand_vals, cand_idx, coords, xf):
    B = all_scores.shape[0]
    S_tabs = _sup_tables(coords)
    rngB = np.arange(B)
    idx_out = np.empty((B, 6), np.int64)

    for g in (0, 1):
        a, b = GROUPS[g]
        s0, e0 = int(OFFS[a]), int(OFFS[b])
        N = N_LIST[g]
        S = S_tabs[g]
        nblk_g = 3
        blk0 = 3 * g  # first extraction block index of this group
        # group-local candidate indices and their exact (divided) values
        gl_idx = np.concatenate([
            cand_idx[:, 8 * (blk0 + j):8 * (blk0 + j) + 8].astype(np.int64)
            + (int(OFFS[a + j]) - s0)
            for j in range(nblk_g)
        ], axis=1)                                           # [B, 24]
        vals = np.take_along_axis(all_scores[:, s0:e0], gl_idx, axis=1)  # [B,24]
        sums_vals = np.concatenate([
            cand_vals[:, 8 * (blk0 + j):8 * (blk0 + j) + 8] for j in range(nblk_g)
        ], axis=1)                                           # [B,24] raw sums

        # order candidates by exact score desc, index asc for ties
        order = np.lexsort((gl_idx, -vals.astype(np.float64)), axis=1)
        o_idx = np.take_along_axis(gl_idx, order, axis=1)
        o_val = np.take_along_axis(vals, order, axis=1)
        o_blk = np.take_along_axis(
            np.broadcast_to(np.repeat(np.arange(3), 8)[None, :], (B, 24)).copy(),
            order, axis=1)

        # duplicate-value candidates within the shipped top-8s -> fallback
        dup_flag = np.zeros((B,), bool)
        for j in range(nblk_g):
            svals = sums_vals[:, 8 * j:8 * j + 8]
            dup_flag |= (np.abs(np.diff(svals, axis=1)) <= 0.0).any(axis=1)

        alive = np.ones((B, 24), bool)
        done = np.zeros((B,), bool)
        nkept = np.zeros((B,), np.int64)
        picks = np.zeros((B, N), np.int64)
        mingap = np.full((B,), np.inf, np.float32)
        for k in range(N):
            mval = np.where(alive, o_val, -np.inf)
            sel = np.argmax(mval, axis=1)                 # first max position
            pv = mval[rngB, sel]
            ok = np.isfinite(pv)
            done |= ~ok
            pidx = o_idx[rngB, sel]
            picks[:, k] = np.where(ok, pidx, 0)
            nkept += ok.astype(np.int64)
            # gap to runner-up (for near-tie flagging)
            m2 = mval.copy()
            m2[rngB, sel] = -np.inf
            runner = np.max(m2, axis=1)
            gap = np.where(np.isfinite(runner), pv - runner, np.inf)
            np.minimum(mingap, gap.astype(np.float32), out=mingap)
            # suppress candidates by IoU with the pick (and the pick itself)
            sup = S[pidx][rngB[:, None], o_idx]           # [B, 24]
            alive &= ~sup
            alive[rngB, sel] = False

        # validity: enough picks, and no extraction block fully eliminated
        elim_per_blk = np.stack(
            [np.sum(~alive & (o_blk == j), axis=1) for j in range(3)], axis=1)
        blk_exhaust = (elim_per_blk >= 8).any(axis=1)
        # picks too close to a block's top-8 floor: the true 9th of that block
        # (not shipped) could tie across the boundary -> route to full fallback
        floors = np.stack(
            [vals[:, 8 * j:8 * j + 8].min(axis=1) for j in range(3)], axis=1)
        maxfloor = np.where(elim_per_blk < 8, floors, -np.inf).max(axis=1)
        pick_vals = np.take_along_axis(
            all_scores[:, s0:e0], picks, axis=1)
        floor_margin = (pick_vals - maxfloor[:, None]).min(axis=1)
        invalid = (nkept < N) | blk_exhaust | dup_flag | (floor_margin < EPS_TIE)
        tie = mingap < EPS_TIE

        fb = invalid & ~tie
        if fb.any():
            fpicks, fgap = _full_nms_group(all_scores[fb, s0:e0], S, N)
            picks[fb] = fpicks
            tie[fb] |= fgap < EPS_TIE
        if tie.any():
            ex = _exact_scores_from_x(xf[tie])
            epicks, _ = _full_nms_group(ex[:, s0:e0], S, N)
            picks[tie] = epicks

        cols = [0, 3, 5][g]
        idx_out[:, cols:cols + N] = picks + s0

    # group 2: plain argmax on the host
    s0, e0 = int(OFFS[6]), int(OFFS[9])
    sl = all_scores[:, s0:e0]
    p = np.argmax(sl, axis=1)
    pv = sl[rngB, p]
    sl2 = sl.copy()
    sl2[rngB, p] = -np.inf
    gap = pv - np.max(sl2, axis=1)
    tie = gap < EPS_TIE
    if tie.any():
        ex = _exact_scores_from_x(xf[tie])
        p[tie] = np.argmax(ex[:, s0:e0], axis=1)
    idx_out[:, 5] = p + s0

    return idx_out


# --------------------------------------------------------------------- kernel

def kernel(x, coords, proposalN):
    x = np.asarray(x)
    coords = np.asarray(coords)
    B = x.shape[0]
    assert B % NCORES == 0 and (B // NCORES) % P == 0, f"unsupported batch {B}"
    assert int(proposalN) == 6
    xf = np.ascontiguousarray(x.reshape(B, NFEAT).astype(np.float32, copy=False))

    M = _build_M()
    Bc = B // NCORES
    nc = _get_kernel(Bc)
    in_maps = [
        {"x": xf[c * Bc:(c + 1) * Bc], "m0": M[0:P], "m1": M[P:NFEAT]}
        for c in range(NCORES)
    ]
    global _LAST_RESULTS
    res = run_bass_kernel_spmd(nc, in_maps, core_ids=list(range(NCORES)), trace=PROFILE)
    _LAST_RESULTS = res
    sums = np.concatenate([r["sums"] for r in res.results], axis=0)
    cand_vals = np.concatenate([r["cand_vals"] for r in res.results], axis=0)
    cand_idx = np.concatenate([r["cand_idx"] for r in res.results], axis=0)

    all_scores = sums / _hw_row()[None, :]

    idx = _host_nms(all_scores, sums, cand_vals, cand_idx, coords, xf)
    idx32 = idx.astype(np.int32)
    s6 = np.take_along_axis(all_scores, idx, axis=1).astype(np.float32)
    return idx32, s6, all_scores


# revision 4
# speedup vs baseline: 5.0516x; 5.0516x over previous
"""Trainium2 Bass kernel for nn_APPM_24111946399794 (nms_detection).

Per batch element (B=65536): 741 multi-scale VALID avgpool window scores from a
1x14x14 map, greedy NMS per 3 ratio groups (3+2+1 picks, IoU<=0.25), returns
(proposalN_indices [B,6] i32, proposalN_windows_scores [B,6] f32,
 window_scores [B,741] f32).

Device (8 NeuronCores, batch-sharded): window SUMS via two fp16 "limb"
matmuls against 0/1 pooling matrices (x = hi + lo*2^-12; the 2^-12 folds into
the lo-pass matrix, all entries exact in fp16; accumulation is fp32 in PSUM),
plus per-block top-8 (value+index) extraction for NMS groups 0/1 on the DVE.
Host: exact fp32 division by window area, candidate walk with precomputed
IoU-suppression tables, vectorized full-NMS fallback for batches whose picks
are not contained in the shipped candidates, and an exact sequential-order
recompute for near-tie batches.
"""

from contextlib import ExitStack

import numpy as np

import concourse.bacc as bacc
import concourse.mybir as mybir
import concourse.tile as tile
from concourse.bass_utils import run_bass_kernel_spmd
from concourse.masks import make_identity

# ---------------------------------------------------------------- problem spec
SIZE = 14
RATIOS = [(4, 4), (3, 5), (5, 3), (6, 6), (5, 7), (7, 5), (8, 8), (6, 10), (10, 6)]
GROUPS = [(0, 3), (3, 6), (6, 9)]
N_LIST = [3, 2, 1]
IOU_THRESHS = [0.25, 0.25, 0.25]
WIN_NUMS = [(SIZE - h + 1) * (SIZE - w + 1) for h, w in RATIOS]
OFFS = np.cumsum([0] + WIN_NUMS)          # [0,121,241,361,442,522,602,651,696,741]
NWIN = int(OFFS[-1])                      # 741
NCORES = 8
P = 128
NFEAT = SIZE * SIZE                       # 196
K2 = NFEAT - P                            # 68
LO_SCALE = 4096.0                         # lo limb scale (2^12)
# device-side top-8 extraction blocks; ratios sharing an h*w divisor merged
# (sums are order-equivalent to scores within a block)
EX_BLOCKS = [(0, 121), (121, 361), (361, 442), (442, 602)]
BLK_PER_GROUP = {0: [(0, 121), (121, 361)], 1: [(0, 81), (81, 241)]}  # group-local
NBLK = len(EX_BLOCKS)
NCAND = 8 * NBLK                          # 32 shipped per batch element
EPS_TIE = 1e-5                            # near-tie flag threshold on scores

_KERNEL_CACHE: dict = {}
PROFILE = False
_LAST_RESULTS = None


def _build_M() -> np.ndarray:
    """[196, 741] 0/1 fp32 pooling-sum matrix; column order matches reference."""
    M = np.zeros((NFEAT, NWIN), np.float32)
    col = 0
    for (h, w) in RATIOS:
        for io in range(SIZE - h + 1):
            for jo in range(SIZE - w + 1):
                blk = np.zeros((SIZE, SIZE), np.float32)
                blk[io:io + h, jo:jo + w] = 1.0
                M[:, col] = blk.reshape(NFEAT)
                col += 1
    return M


def _hw_row() -> np.ndarray:
    """[741] fp32 window areas (h*w) per window for the final division."""
    return np.concatenate([
        np.full((WIN_NUMS[i],), np.float32(h * w), np.float32)
        for i, (h, w) in enumerate(RATIOS)
    ])


def _build_device_kernel(Bc: int):
    """Bass kernel for one core processing Bc batch rows."""
    assert Bc % P == 0
    ntiles = Bc // P
    nc = bacc.Bacc("TRN2", target_bir_lowering=False, debug=False)
    f32 = mybir.dt.float32
    f16 = mybir.dt.float16
    u32 = mybir.dt.uint32

    xh_d = nc.dram_tensor("x_hi", [Bc, NFEAT], f16, kind="ExternalInput").ap()
    xl_d = nc.dram_tensor("x_lo", [Bc, NFEAT], f16, kind="ExternalInput").ap()
    mh_d = nc.dram_tensor("m_hi", [NFEAT, NWIN], f16, kind="ExternalInput").ap()
    ml_d = nc.dram_tensor("m_lo", [NFEAT, NWIN], f16, kind="ExternalInput").ap()
    sums_d = nc.dram_tensor("sums", [Bc, NWIN], f32, kind="ExternalOutput").ap()
    cand_d = nc.dram_tensor("cand", [Bc, 2 * NCAND], u32, kind="ExternalOutput").ap()

    with tile.TileContext(nc) as tc, ExitStack() as ctx:
        const = ctx.enter_context(tc.tile_pool(name="const", bufs=1))
        xin = ctx.enter_context(tc.tile_pool(name="xin", bufs=4))
        xtp = ctx.enter_context(tc.tile_pool(name="xtp", bufs=2, space="PSUM"))
        xts = ctx.enter_context(tc.tile_pool(name="xts", bufs=2))
        sps = ctx.enter_context(tc.tile_pool(name="sps", bufs=3, space="PSUM"))
        ssb = ctx.enter_context(tc.tile_pool(name="ssb", bufs=3))
        cnd = ctx.enter_context(tc.tile_pool(name="cnd", bufs=3))

        mh_sb = const.tile([P, 2, NWIN], f16)      # [k-chunk partitions, chunk, win]
        nc.sync.dma_start(mh_sb[:, 0, :], mh_d[0:P, :])
        nc.sync.dma_start(mh_sb[0:K2, 1, :], mh_d[P:NFEAT, :])
        ml_sb = const.tile([P, 2, NWIN], f16)
        nc.sync.dma_start(ml_sb[:, 0, :], ml_d[0:P, :])
        nc.sync.dma_start(ml_sb[0:K2, 1, :], ml_d[P:NFEAT, :])
        ident = const.tile([P, P], f16)
        make_identity(nc, ident[:])

        for t in range(ntiles):
            rows = slice(t * P, (t + 1) * P)

            xh_sb = xin.tile([P, NFEAT], f16)
            nc.sync.dma_start(xh_sb[:], xh_d[rows, :])
            xl_sb = xin.tile([P, NFEAT], f16)
            nc.gpsimd.dma_start(xl_sb[:], xl_d[rows, :])

            # transpose both limbs -> one fp16 PSUM tile, one ACT copy out
            xt_ps = xtp.tile([P, 4 * P], f16)
            nc.tensor.transpose(xt_ps[:, 0 * P:1 * P], xh_sb[:, 0:P], ident[:])
            nc.tensor.transpose(xt_ps[0:K2, 1 * P:2 * P], xh_sb[:, P:NFEAT], ident[:])
            nc.tensor.transpose(xt_ps[:, 2 * P:3 * P], xl_sb[:, 0:P], ident[:])
            nc.tensor.transpose(xt_ps[0:K2, 3 * P:4 * P], xl_sb[:, P:NFEAT], ident[:])
            xt_sb = xts.tile([P, 4 * P], f16)
            nc.scalar.copy(xt_sb[:], xt_ps[:])

            lhs_hi0 = xt_sb[:, 0 * P:1 * P]
            lhs_hi1 = xt_sb[0:K2, 1 * P:2 * P]
            lhs_lo0 = xt_sb[:, 2 * P:3 * P]
            lhs_lo1 = xt_sb[0:K2, 3 * P:4 * P]

            # window sums [128b, 741]: hi & lo limb passes accumulate in PSUM
            s0 = sps.tile([P, 512], f32)
            s1 = sps.tile([P, NWIN - 512], f32)
            nc.tensor.matmul(s0[:], lhs_hi0, mh_sb[:, 0, 0:512], start=True, stop=False)
            nc.tensor.matmul(s1[:], lhs_hi0, mh_sb[:, 0, 512:NWIN], start=True, stop=False)
            nc.tensor.matmul(s0[:], lhs_hi1, mh_sb[0:K2, 1, 0:512], start=False, stop=False)
            nc.tensor.matmul(s1[:], lhs_hi1, mh_sb[0:K2, 1, 512:NWIN], start=False, stop=False)
            nc.tensor.matmul(s0[:], lhs_lo0, ml_sb[:, 0, 0:512], start=False, stop=False)
            nc.tensor.matmul(s1[:], lhs_lo0, ml_sb[:, 0, 512:NWIN], start=False, stop=False)
            nc.tensor.matmul(s0[:], lhs_lo1, ml_sb[0:K2, 1, 0:512], start=False, stop=True)
            nc.tensor.matmul(s1[:], lhs_lo1, ml_sb[0:K2, 1, 512:NWIN], start=False, stop=True)

            sums_sb = ssb.tile([P, NWIN], f32)
            nc.scalar.copy(sums_sb[:, 0:512], s0[:])
            nc.scalar.copy(sums_sb[:, 512:NWIN], s1[:])

            # per-block top-8 values + indices (groups 0 and 1), one packed tile
            cd_sb = cnd.tile([P, 2 * NCAND], u32)
            cv_view = cd_sb[:, 0:NCAND].bitcast(mybir.dt.float32)
            for j, (s, e) in enumerate(EX_BLOCKS):
                nc.vector.max(out=cv_view[:, 8 * j:8 * j + 8], in_=sums_sb[:, s:e])
                nc.vector.max_index(
                    out=cd_sb[:, NCAND + 8 * j:NCAND + 8 * j + 8],
                    in_max=cv_view[:, 8 * j:8 * j + 8],
                    in_values=sums_sb[:, s:e],
                )

            nc.sync.dma_start(sums_d[rows, :], sums_sb[:])
            nc.gpsimd.dma_start(cand_d[rows, :], cd_sb[:])

    nc.compile()
    return nc


def _get_kernel(Bc: int):
    if Bc not in _KERNEL_CACHE:
        _KERNEL_CACHE[Bc] = _build_device_kernel(Bc)
    return _KERNEL_CACHE[Bc]


# ---------------------------------------------------------------- host helpers

def _sup_tables(coords: np.ndarray):
    """Per-group boolean suppression tables S[i, j] = IoU(i, j) > thresh."""
    cf = coords.astype(np.float32)
    areas = (cf[:, 2] - cf[:, 0] + 1.0) * (cf[:, 3] - cf[:, 1] + 1.0)
    tabs = []
    for g, (a, b) in enumerate(GROUPS):
        s, e = int(OFFS[a]), int(OFFS[b])
        c = cf[s:e]
        ar = areas[s:e]
        lx = np.minimum(c[None, :, 2], c[:, None, 2]) - np.maximum(c[None, :, 0], c[:, None, 0]) + 1.0
        ly = np.minimum(c[None, :, 3], c[:, None, 3]) - np.maximum(c[None, :, 1], c[:, None, 1]) + 1.0
        inter = np.where((lx < 0) | (ly < 0), 0.0, lx * ly)
        iou = inter / (ar[None, :] + ar[:, None] - inter)
        tabs.append(iou > IOU_THRESHS[g])
    return tabs


def _full_nms_group(scores_g: np.ndarray, S: np.ndarray, N: int):
    """Vectorized greedy NMS over a [n, W] slice. Returns picks [n, N] and the
    min top1-top2 gap across steps (for near-tie flagging)."""
    n = scores_g.shape[0]
    cur = scores_g.astype(np.float32).copy()
    picks = np.empty((n, N), np.int64)
    mingap = np.full((n,), np.inf, np.float32)
    rng = np.arange(n)
    for k in range(N):
        p = np.argmax(cur, axis=1)
        picks[:, k] = p
        pv = cur[rng, p]
        cur[rng, p] = -np.inf
        runner = np.max(cur, axis=1)
        with np.errstate(invalid="ignore"):
            gap = np.where(np.isfinite(runner), pv - runner, np.inf)
        np.minimum(mingap, gap.astype(np.float32), out=mingap)
        if k + 1 < N:
            cur[S[p]] = -np.inf
    return picks, mingap


def _exact_scores_from_x(xf: np.ndarray) -> np.ndarray:
    """Reference-bitexact scores for a (small) batch subset: sequential fp32
    accumulation over window elements in row-major order, then fp32 divide."""
    M = _build_M()
    n = xf.shape[0]
    acc = np.zeros((n, NWIN), np.float32)
    for k in range(NFEAT):
        acc += xf[:, k:k + 1] * M[k][None, :]
    return acc / _hw_row()[None, :]


def _host_nms(all_scores, cand_vals, cand_idx, coords, xf):
    B = all_scores.shape[0]
    S_tabs = _sup_tables(coords)
    rngB = np.arange(B)
    idx_out = np.empty((B, 6), np.int64)

    for g in (0, 1):
        a, b = GROUPS[g]
        s0, e0 = int(OFFS[a]), int(OFFS[b])
        N = N_LIST[g]
        S = S_tabs[g]
        blocks = BLK_PER_GROUP[g]
        blk0 = 2 * g                       # first extraction block of this group
        ncand_g = 8 * len(blocks)          # 16
        gl_idx = np.concatenate([
            cand_idx[:, 8 * (blk0 + j):8 * (blk0 + j) + 8].astype(np.int64)
            + blocks[j][0]
            for j in range(len(blocks))
        ], axis=1)                                           # [B, 16] group-local
        vals = np.take_along_axis(all_scores[:, s0:e0], gl_idx, axis=1)
        sums_vals = np.concatenate([
            cand_vals[:, 8 * (blk0 + j):8 * (blk0 + j) + 8]
            for j in range(len(blocks))
        ], axis=1)

        # order candidates by exact score desc, index asc for ties
        order = np.lexsort((gl_idx, -vals.astype(np.float64)), axis=1)
        o_idx = np.take_along_axis(gl_idx, order, axis=1)
        o_val = np.take_along_axis(vals, order, axis=1)
        o_blk = np.take_along_axis(
            np.broadcast_to(
                np.repeat(np.arange(len(blocks)), 8)[None, :], (B, ncand_g)
            ).copy(),
            order, axis=1)

        # duplicate raw-sum values within a shipped top-8 -> fallback
        dup_flag = np.zeros((B,), bool)
        for j in range(len(blocks)):
            svals = sums_vals[:, 8 * j:8 * j + 8]
            dup_flag |= (np.diff(svals, axis=1) == 0.0).any(axis=1)

        alive = np.ones((B, ncand_g), bool)
        nkept = np.zeros((B,), np.int64)
        picks = np.zeros((B, N), np.int64)
        mingap = np.full((B,), np.inf, np.float32)
        for k in range(N):
            mval = np.where(alive, o_val, -np.inf)
            sel = np.argmax(mval, axis=1)
            pv = mval[rngB, sel]
            ok = np.isfinite(pv)
            pidx = o_idx[rngB, sel]
            picks[:, k] = np.where(ok, pidx, 0)
            nkept += ok.astype(np.int64)
            m2 = mval.copy()
            m2[rngB, sel] = -np.inf
            runner = np.max(m2, axis=1)
            with np.errstate(invalid="ignore"):
                gap = np.where(np.isfinite(runner) & ok, pv - runner, np.inf)
            np.minimum(mingap, gap.astype(np.float32), out=mingap)
            sup = S[pidx][rngB[:, None], o_idx]
            alive &= ~sup
            alive[rngB, sel] = False

        # validity: enough picks, no block fully eliminated, picks clear of
        # any live block's top-8 floor (the unseen 9th could tie across it)
        elim_per_blk = np.stack(
            [np.sum(~alive & (o_blk == j), axis=1) for j in range(len(blocks))],
            axis=1)
        blk_exhaust = (elim_per_blk >= 8).any(axis=1)
        floors = np.stack(
            [vals[:, 8 * j:8 * j + 8].min(axis=1) for j in range(len(blocks))],
            axis=1)
        maxfloor = np.where(elim_per_blk < 8, floors, -np.inf).max(axis=1)
        pick_vals = np.take_along_axis(all_scores[:, s0:e0], picks, axis=1)
        floor_margin = (pick_vals - maxfloor[:, None]).min(axis=1)
        invalid = (nkept < N) | blk_exhaust | dup_flag | (floor_margin < EPS_TIE)
        tie = mingap < EPS_TIE

        fb = invalid & ~tie
        if fb.any():
            fpicks, fgap = _full_nms_group(all_scores[fb, s0:e0], S, N)
            picks[fb] = fpicks
            tie2 = np.zeros((B,), bool)
            tie2[np.nonzero(fb)[0]] = fgap < EPS_TIE
            tie |= tie2
        if tie.any():
            ex = _exact_scores_from_x(xf[tie])
            epicks, _ = _full_nms_group(ex[:, s0:e0], S, N)
            picks[tie] = epicks

        cols = [0, 3, 5][g]
        idx_out[:, cols:cols + N] = picks + s0

    # group 2: plain argmax on the host
    s0, e0 = int(OFFS[6]), int(OFFS[9])
    sl = all_scores[:, s0:e0]
    p = np.argmax(sl, axis=1)
    pv = sl[rngB, p]
    sl2 = sl.copy()
    sl2[rngB, p] = -np.inf
    gap = pv - np.max(sl2, axis=1)
    tie = gap < EPS_TIE
    if tie.any():
        ex = _exact_scores_from_x(xf[tie])
        p[tie] = np.argmax(ex[:, s0:e0], axis=1)
    idx_out[:, 5] = p + s0

    return idx_out


# --------------------------------------------------------------------- kernel

def kernel(x, coords, proposalN):
    x = np.asarray(x)
    coords = np.asarray(coords)
    B = x.shape[0]
    assert B % NCORES == 0 and (B // NCORES) % P == 0, f"unsupported batch {B}"
    assert int(proposalN) == 6
    xf = np.ascontiguousarray(x.reshape(B, NFEAT).astype(np.float32, copy=False))

    # fp16 limb split: x = hi + lo/4096 with |x - (hi + lo/4096)| <~ 2^-22 |x|
    x_hi = xf.astype(np.float16)
    resid = xf - x_hi.astype(np.float32)
    x_lo = (resid * LO_SCALE).astype(np.float16)

    M = _build_M()
    m_hi = M.astype(np.float16)                       # 0/1, exact
    m_lo = (M * np.float32(1.0 / LO_SCALE)).astype(np.float16)  # 2^-12, exact

    Bc = B // NCORES
    nc = _get_kernel(Bc)
    in_maps = [
        {"x_hi": x_hi[c * Bc:(c + 1) * Bc], "x_lo": x_lo[c * Bc:(c + 1) * Bc],
         "m_hi": m_hi, "m_lo": m_lo}
        for c in range(NCORES)
    ]
    global _LAST_RESULTS
    res = run_bass_kernel_spmd(nc, in_maps, core_ids=list(range(NCORES)), trace=PROFILE)
    _LAST_RESULTS = res
    sums = np.concatenate([r["sums"] for r in res.results], axis=0)
    cand = np.concatenate([r["cand"] for r in res.results], axis=0)
    cand_vals = cand[:, 0:NCAND].view(np.float32)
    cand_idx = cand[:, NCAND:2 * NCAND]

    all_scores = sums / _hw_row()[None, :]

    idx = _host_nms(all_scores, cand_vals, cand_idx, coords, xf)
    idx32 = idx.astype(np.int32)
    s6 = np.take_along_axis(all_scores, idx, axis=1).astype(np.float32)
    return idx32, s6, all_scores
